# revision 36
# baseline (speedup 1.0000x reference)
"""AttnNet kernel for Trainium2: attn = softmax(einsum("bsh,bh->bs", facts, questions))[:, None, :].

Full shapes: questions [64, 4096] f32, facts [64, 512, 4096] f32 -> out [64, 1, 512] f32.
Data-parallel over batch: 8 batches per NeuronCore x 8 cores, no collectives.

Per-core dataflow v2 (B_LOC=8, S=512, H=4096):
  - facts streamed as 32 [128(s), 4096(h)] f32 tiles (2 MiB), alternating between the
    two HWDGE rings (nc.sync / nc.scalar) so per-DMA fixed costs hide behind each other.
  - q[b] broadcast to 128 partitions via PE outer-product (ones[1,128]^T @ q_row[1,512]
    per PSUM bank) + ACT copy to SBUF; PE/ACT are otherwise idle, gpsimd unused.
  - One fused DVE op per tile: scalar_tensor_tensor(out=bf16 dummy, in0=ftile,
    op0=bypass, op1=mult, in1=q_b, accum_out=E[:, col]) -> multiply + row-sum in a
    single pass (accumulator is fp32 internally; bf16 dummy halves write traffic).
  - Epilogue per batch: PE-transpose E[:,4b:4b+4] -> PSUM [4,128], ACT copy into
    e_t[32,128]. After batches 3 and 7: regroup rows to [4,512] via SWDGE SBUF->SBUF
    DMA, then softmax (DVE max / ACT exp+sum / DVE recip+scale), out DMA on scalar ring.
"""

import numpy as np

B, S, H = 64, 512, 4096
N_CORES = 8
B_LOC = B // N_CORES  # 8
P = 128
SC = S // P  # 4 s-chunks per batch
NB = 512  # f32 elems per PSUM bank

_CACHE = {}


def _build_bass():
    import concourse.bacc as bacc
    import concourse.mybir as mybir
    import concourse.tile as tile
    from concourse.masks import make_identity

    f32 = mybir.dt.float32
    bf16 = mybir.dt.bfloat16

    nc = bacc.Bacc("TRN2", target_bir_lowering=False, debug=False)
    facts = nc.dram_tensor("facts", [B_LOC, S, H], f32, kind="ExternalInput").ap()
    questions = nc.dram_tensor("questions", [B_LOC, H], f32, kind="ExternalInput").ap()
    ind_in = nc.dram_tensor("ind", [B_LOC, B_LOC * P], f32, kind="ExternalInput").ap()
    attn = nc.dram_tensor("attn", [B_LOC, S], f32, kind="ExternalOutput").ap()

    with tile.TileContext(nc) as tc:
        with (
            tc.tile_pool(name="consts", bufs=1) as consts,
            tc.tile_pool(name="fpa", bufs=4) as fpa,
            tc.tile_pool(name="fpb", bufs=4) as fpb,
            tc.tile_pool(name="qsb", bufs=2) as qsb,
            tc.tile_pool(name="smp", bufs=2) as smp,
            tc.tile_pool(name="qps", bufs=2, space="PSUM") as qps,
            tc.tile_pool(name="erps", bufs=2, space="PSUM") as erps,
        ):
            # gpsimd queue order matters at startup: q_rows first, then the
            # batch-0 broadcast (DVE is still idle, so gpsimd SBUF-write
            # contention is free), then ind + identity (needed much later).
            # q_rows + ind go on the sync HWDGE ring at high priority (ahead of
            # the 2 MiB facts DMAs). Keeping them off gpsimd/SWDGE matters: any
            # pending SWDGE DMA forces an ~12us DRAIN before PartitionBroadcast.
            q_rows = consts.tile([B_LOC, H], f32)
            with tc.high_priority():
                nc.sync.dma_start(out=q_rows[:], in_=questions)

            # ind[:, b*128:(b+1)*128] is the [8, 128] selector for batch b:
            # row b ones, rest zero -> matmul(ind_b, q_rows) broadcasts q[b].
            ind = consts.tile([B_LOC, B_LOC * P], f32)

            # energies: column b*SC+c holds energies[b, c*128:(c+1)*128] on partitions
            E = consts.tile([P, B_LOC * SC], f32)
            dummy = consts.tile([P, H], bf16)

            def emit_q_broadcast(b):
                """Broadcast q[b] to [128, H].

                Batch 0 uses gpsimd partition_broadcast: DVE is still idle at
                startup, so gpsimd's SBUF-write contention is free and q_b(0)
                is ready ~15 us earlier than the PE chain could deliver it.
                Later batches use PE outer-product + ACT copies via PSUM;
                gpsimd broadcast there would degrade the critical DVE op
                ~4.4 -> ~5.7 us (measured).
                """
                q_b = qsb.tile([P, H], f32)
                if b == 0:
                    # high priority: the scheduler otherwise defers this ~13us,
                    # and the first DVE op waits on it
                    with tc.high_priority():
                        nc.gpsimd.partition_broadcast(q_b[:], q_rows[0:1, :])
                    return q_b
                for k in range(H // NB):
                    ps = qps.tile([P, NB], f32)
                    nc.tensor.matmul(
                        ps[:],
                        ind[:, b * P : (b + 1) * P],
                        q_rows[:, k * NB : (k + 1) * NB],
                        start=True,
                        stop=True,
                    )
                    nc.scalar.copy(q_b[:, k * NB : (k + 1) * NB], ps[:])
                return q_b

            q_b0 = emit_q_broadcast(0)
            with tc.high_priority():
                nc.sync.dma_start(out=ind[:], in_=ind_in)
            identity = consts.tile([P, P], f32)
            make_identity(nc, identity[:])

            def emit_group_softmax(g):
                """softmax + output store for batches [4g, 4g+4).

                PE transposes strided E column-slices straight into row layout:
                er_ps[b_g, c*128+i] = E[i, (4g+b_g)*SC+c], so no regroup DMA.
                """
                er_ps = erps.tile([SC, S], f32)
                Ev = E[:].rearrange("p (b c) -> p c b", c=SC)  # [128, c, b]
                for c in range(SC):
                    nc.tensor.transpose(
                        er_ps[:, c * P : (c + 1) * P],
                        Ev[:, c, 4 * g : 4 * g + 4],
                        identity[:],
                    )
                # max (DVE) and exp (ACT) read the PSUM tile directly; no copy
                nmax = smp.tile([SC, 1], f32)
                nc.vector.reduce_max(
                    nmax[:], er_ps[:], axis=mybir.AxisListType.X, negate=True
                )
                pexp = smp.tile([SC, S], f32)
                dn = smp.tile([SC, 1], f32)
                nc.scalar.activation(
                    pexp[:],
                    er_ps[:],
                    mybir.ActivationFunctionType.Exp,
                    bias=nmax[:],
                    scale=1.0,
                    accum_out=dn[:],
                )
                rc = smp.tile([SC, 1], f32)
                nc.vector.reciprocal(rc[:], dn[:])
                at = smp.tile([SC, S], f32)
                nc.vector.tensor_scalar_mul(at[:], pexp[:], rc[:])
                nc.scalar.dma_start(out=attn[4 * g : 4 * g + 4, :], in_=at[:])

            q_cur = q_b0
            del q_b0
            for b in range(B_LOC):
                ftiles = []
                for c in range(SC):
                    t = b * SC + c
                    pool, eng = (fpa, nc.sync) if t % 2 == 0 else (fpb, nc.scalar)
                    ftile = pool.tile([P, H], f32)
                    eng.dma_start(out=ftile[:], in_=facts[b, c * P : (c + 1) * P, :])
                    ftiles.append(ftile)
                for c in range(SC):
                    col = b * SC + c
                    # fused multiply + row-sum on DVE; dummy bf16 out (values unused)
                    nc.vector.scalar_tensor_tensor(
                        out=dummy[:],
                        in0=ftiles[c][:],
                        scalar=1.0,
                        in1=q_cur[:],
                        op0=mybir.AluOpType.bypass,
                        op1=mybir.AluOpType.mult,
                        accum_out=E[:, col : col + 1],
                    )
                if b + 1 < B_LOC:
                    q_next = emit_q_broadcast(b + 1)
                else:
                    q_next = None
                if b == 3:
                    emit_group_softmax(0)
                q_cur = q_next
            emit_group_softmax(1)

    nc.compile()
    return nc


def _get_nc():
    if "nc" not in _CACHE:
        _CACHE["nc"] = _build_bass()
    return _CACHE["nc"]


def _shard_inputs(questions, facts):
    questions = np.ascontiguousarray(np.asarray(questions), dtype=np.float32)
    facts = np.ascontiguousarray(np.asarray(facts), dtype=np.float32)
    ind = np.zeros((B_LOC, B_LOC * P), dtype=np.float32)
    for b in range(B_LOC):
        ind[b, b * P : (b + 1) * P] = 1.0
    in_maps = []
    for i in range(N_CORES):
        sl = slice(i * B_LOC, (i + 1) * B_LOC)
        in_maps.append(
            {
                "facts": np.ascontiguousarray(facts[sl]),
                "questions": np.ascontiguousarray(questions[sl]),
                "ind": ind,
            }
        )
    return in_maps


def _run(questions, facts, **run_kwargs):
    from concourse.bass_utils import run_bass_kernel_spmd

    nc = _get_nc()
    in_maps = _shard_inputs(questions, facts)
    res = run_bass_kernel_spmd(nc, in_maps, core_ids=list(range(N_CORES)), **run_kwargs)
    out = np.stack([np.asarray(res.results[i]["attn"]) for i in range(N_CORES)])
    return out.reshape(B, S)[:, None, :].astype(np.float32), res


def kernel(questions, facts):
    out, _ = _run(questions, facts)
    return out


# revision 41
# speedup vs baseline: 1.0013x; 1.0013x over previous
"""AttnNet kernel for Trainium2: attn = softmax(einsum("bsh,bh->bs", facts, questions))[:, None, :].

Full shapes: questions [64, 4096] f32, facts [64, 512, 4096] f32 -> out [64, 1, 512] f32.
Data-parallel over batch: 8 batches per NeuronCore x 8 cores, no collectives.

Per-core dataflow v2 (B_LOC=8, S=512, H=4096):
  - facts streamed as 32 [128(s), 4096(h)] f32 tiles (2 MiB), alternating between the
    two HWDGE rings (nc.sync / nc.scalar) so per-DMA fixed costs hide behind each other.
  - q[b] broadcast to 128 partitions via PE outer-product (ones[1,128]^T @ q_row[1,512]
    per PSUM bank) + ACT copy to SBUF; PE/ACT are otherwise idle, gpsimd unused.
  - One fused DVE op per tile: scalar_tensor_tensor(out=bf16 dummy, in0=ftile,
    op0=bypass, op1=mult, in1=q_b, accum_out=E[:, col]) -> multiply + row-sum in a
    single pass (accumulator is fp32 internally; bf16 dummy halves write traffic).
  - Epilogue per batch: PE-transpose E[:,4b:4b+4] -> PSUM [4,128], ACT copy into
    e_t[32,128]. After batches 3 and 7: regroup rows to [4,512] via SWDGE SBUF->SBUF
    DMA, then softmax (DVE max / ACT exp+sum / DVE recip+scale), out DMA on scalar ring.
"""

import numpy as np

B, S, H = 64, 512, 4096
N_CORES = 8
B_LOC = B // N_CORES  # 8
P = 128
SC = S // P  # 4 s-chunks per batch
NB = 512  # f32 elems per PSUM bank

_CACHE = {}


def _build_bass():
    import concourse.bacc as bacc
    import concourse.mybir as mybir
    import concourse.tile as tile
    from concourse.masks import make_identity

    f32 = mybir.dt.float32
    bf16 = mybir.dt.bfloat16

    nc = bacc.Bacc("TRN2", target_bir_lowering=False, debug=False)
    facts = nc.dram_tensor("facts", [B_LOC, S, H], f32, kind="ExternalInput").ap()
    questions = nc.dram_tensor("questions", [B_LOC, H], f32, kind="ExternalInput").ap()
    ind_in = nc.dram_tensor("ind", [B_LOC, B_LOC * P], f32, kind="ExternalInput").ap()
    attn = nc.dram_tensor("attn", [B_LOC, S], f32, kind="ExternalOutput").ap()

    with tile.TileContext(nc) as tc:
        with (
            tc.tile_pool(name="consts", bufs=1) as consts,
            tc.tile_pool(name="fpa", bufs=4) as fpa,
            tc.tile_pool(name="fpb", bufs=3) as fpb,
            tc.tile_pool(name="qsb", bufs=2) as qsb,
            tc.tile_pool(name="smp", bufs=2) as smp,
            tc.tile_pool(name="qps", bufs=2, space="PSUM") as qps,
            tc.tile_pool(name="erps", bufs=2, space="PSUM") as erps,
        ):
            # gpsimd queue order matters at startup: q_rows first, then the
            # batch-0 broadcast (DVE is still idle, so gpsimd SBUF-write
            # contention is free), then ind + identity (needed much later).
            # Batch-0 q row as a single-partition load: a [1, H] transfer only
            # touches an early-starting SDMA engine, so it completes ~8us in,
            # while partition-spread transfers are gated by straggler engines
            # (~18us). Keeping it off gpsimd/SWDGE also matters: any pending
            # SWDGE DMA forces an ~12us DRAIN before PartitionBroadcast.
            q_row0 = consts.tile([1, H], f32)
            with tc.high_priority():
                nc.sync.dma_start(out=q_row0[:], in_=questions[0:1, :])

            # ind[:, b*128:(b+1)*128] is the [8, 128] selector for batch b:
            # row b ones, rest zero -> matmul(ind_b, q_rows) broadcasts q[b].
            ind = consts.tile([B_LOC, B_LOC * P], f32)

            # energies: column b*SC+c holds energies[b, c*128:(c+1)*128] on partitions
            E = consts.tile([P, B_LOC * SC], f32)
            dummy = consts.tile([P, H], bf16)

            def emit_q_broadcast(b):
                """Broadcast q[b] to [128, H].

                Batch 0 uses gpsimd partition_broadcast: DVE is still idle at
                startup, so gpsimd's SBUF-write contention is free and q_b(0)
                is ready ~15 us earlier than the PE chain could deliver it.
                Later batches use PE outer-product + ACT copies via PSUM;
                gpsimd broadcast there would degrade the critical DVE op
                ~4.4 -> ~5.7 us (measured).
                """
                q_b = qsb.tile([P, H], f32)
                if b == 0:
                    # high priority: the scheduler otherwise defers this ~13us,
                    # and the first DVE op waits on it
                    with tc.high_priority():
                        nc.gpsimd.partition_broadcast(q_b[:], q_row0[:])
                    return q_b
                for k in range(H // NB):
                    ps = qps.tile([P, NB], f32)
                    nc.tensor.matmul(
                        ps[:],
                        ind[:, b * P : (b + 1) * P],
                        q_rows[:, k * NB : (k + 1) * NB],
                        start=True,
                        stop=True,
                    )
                    nc.scalar.copy(q_b[:, k * NB : (k + 1) * NB], ps[:])
                return q_b

            q_b0 = emit_q_broadcast(0)
            # q_rows (for the PE broadcast path, batches 1-7) and ind at normal
            # priority: they are straggler-gated (~18us) but only needed ~22us in
            q_rows = consts.tile([B_LOC, H], f32)
            nc.sync.dma_start(out=q_rows[:], in_=questions)
            nc.sync.dma_start(out=ind[:], in_=ind_in)
            identity = consts.tile([P, P], f32)
            make_identity(nc, identity[:])

            def emit_group_softmax(g):
                """softmax + output store for batches [4g, 4g+4).

                PE transposes strided E column-slices straight into row layout:
                er_ps[b_g, c*128+i] = E[i, (4g+b_g)*SC+c], so no regroup DMA.
                """
                er_ps = erps.tile([SC, S], f32)
                Ev = E[:].rearrange("p (b c) -> p c b", c=SC)  # [128, c, b]
                for c in range(SC):
                    nc.tensor.transpose(
                        er_ps[:, c * P : (c + 1) * P],
                        Ev[:, c, 4 * g : 4 * g + 4],
                        identity[:],
                    )
                # max (DVE) and exp (ACT) read the PSUM tile directly; no copy
                nmax = smp.tile([SC, 1], f32)
                nc.vector.reduce_max(
                    nmax[:], er_ps[:], axis=mybir.AxisListType.X, negate=True
                )
                pexp = smp.tile([SC, S], f32)
                dn = smp.tile([SC, 1], f32)
                nc.scalar.activation(
                    pexp[:],
                    er_ps[:],
                    mybir.ActivationFunctionType.Exp,
                    bias=nmax[:],
                    scale=1.0,
                    accum_out=dn[:],
                )
                rc = smp.tile([SC, 1], f32)
                nc.vector.reciprocal(rc[:], dn[:])
                at = smp.tile([SC, S], f32)
                nc.vector.tensor_scalar_mul(at[:], pexp[:], rc[:])
                nc.scalar.dma_start(out=attn[4 * g : 4 * g + 4, :], in_=at[:])

            q_cur = q_b0
            del q_b0
            for b in range(B_LOC):
                ftiles = []
                for c in range(SC):
                    t = b * SC + c
                    pool, eng = (fpa, nc.sync) if t % 2 == 0 else (fpb, nc.scalar)
                    ftile = pool.tile([P, H], f32)
                    if t == 0:
                        # split the first tile across both rings so the first
                        # DVE op can start ~5us earlier
                        nc.sync.dma_start(
                            out=ftile[:, : H // 2],
                            in_=facts[b, c * P : (c + 1) * P, : H // 2],
                        )
                        nc.scalar.dma_start(
                            out=ftile[:, H // 2 :],
                            in_=facts[b, c * P : (c + 1) * P, H // 2 :],
                        )
                    else:
                        eng.dma_start(
                            out=ftile[:], in_=facts[b, c * P : (c + 1) * P, :]
                        )
                    ftiles.append(ftile)
                for c in range(SC):
                    col = b * SC + c
                    # fused multiply + row-sum on DVE; dummy bf16 out (values unused)
                    nc.vector.scalar_tensor_tensor(
                        out=dummy[:],
                        in0=ftiles[c][:],
                        scalar=1.0,
                        in1=q_cur[:],
                        op0=mybir.AluOpType.bypass,
                        op1=mybir.AluOpType.mult,
                        accum_out=E[:, col : col + 1],
                    )
                if b + 1 < B_LOC:
                    q_next = emit_q_broadcast(b + 1)
                else:
                    q_next = None
                if b == 3:
                    emit_group_softmax(0)
                q_cur = q_next
            emit_group_softmax(1)

    nc.compile()
    return nc


def _get_nc():
    if "nc" not in _CACHE:
        _CACHE["nc"] = _build_bass()
    return _CACHE["nc"]


def _shard_inputs(questions, facts):
    questions = np.ascontiguousarray(np.asarray(questions), dtype=np.float32)
    facts = np.ascontiguousarray(np.asarray(facts), dtype=np.float32)
    ind = np.zeros((B_LOC, B_LOC * P), dtype=np.float32)
    for b in range(B_LOC):
        ind[b, b * P : (b + 1) * P] = 1.0
    in_maps = []
    for i in range(N_CORES):
        sl = slice(i * B_LOC, (i + 1) * B_LOC)
        in_maps.append(
            {
                "facts": np.ascontiguousarray(facts[sl]),
                "questions": np.ascontiguousarray(questions[sl]),
                "ind": ind,
            }
        )
    return in_maps


def _run(questions, facts, **run_kwargs):
    from concourse.bass_utils import run_bass_kernel_spmd

    nc = _get_nc()
    in_maps = _shard_inputs(questions, facts)
    res = run_bass_kernel_spmd(nc, in_maps, core_ids=list(range(N_CORES)), **run_kwargs)
    out = np.stack([np.asarray(res.results[i]["attn"]) for i in range(N_CORES)])
    return out.reshape(B, S)[:, None, :].astype(np.float32), res


def kernel(questions, facts):
    out, _ = _run(questions, facts)
    return out


# revision 45
# speedup vs baseline: 1.0546x; 1.0532x over previous
"""AttnNet kernel for Trainium2: attn = softmax(einsum("bsh,bh->bs", facts, questions))[:, None, :].

Full shapes: questions [64, 4096] f32, facts [64, 512, 4096] f32 -> out [64, 1, 512] f32.
Data-parallel over batch: 8 batches per NeuronCore x 8 cores, no collectives.

Per-core dataflow v2 (B_LOC=8, S=512, H=4096):
  - facts streamed as 32 [128(s), 4096(h)] f32 tiles (2 MiB), alternating between the
    two HWDGE rings (nc.sync / nc.scalar) so per-DMA fixed costs hide behind each other.
  - q[b] broadcast to 128 partitions via PE outer-product (ones[1,128]^T @ q_row[1,512]
    per PSUM bank) + ACT copy to SBUF; PE/ACT are otherwise idle, gpsimd unused.
  - One fused DVE op per tile: scalar_tensor_tensor(out=bf16 dummy, in0=ftile,
    op0=bypass, op1=mult, in1=q_b, accum_out=E[:, col]) -> multiply + row-sum in a
    single pass (accumulator is fp32 internally; bf16 dummy halves write traffic).
  - Epilogue per batch: PE-transpose E[:,4b:4b+4] -> PSUM [4,128], ACT copy into
    e_t[32,128]. After batches 3 and 7: regroup rows to [4,512] via SWDGE SBUF->SBUF
    DMA, then softmax (DVE max / ACT exp+sum / DVE recip+scale), out DMA on scalar ring.
"""

import numpy as np

B, S, H = 64, 512, 4096
N_CORES = 8
B_LOC = B // N_CORES  # 8
P = 128
SC = S // P  # 4 s-chunks per batch
NB = 512  # f32 elems per PSUM bank

_CACHE = {}


def _build_bass():
    import concourse.bacc as bacc
    import concourse.mybir as mybir
    import concourse.tile as tile
    from concourse.masks import make_identity

    f32 = mybir.dt.float32
    bf16 = mybir.dt.bfloat16

    nc = bacc.Bacc("TRN2", target_bir_lowering=False, debug=False)
    facts = nc.dram_tensor("facts", [B_LOC, S, H], f32, kind="ExternalInput").ap()
    questions = nc.dram_tensor("questions", [B_LOC, H], f32, kind="ExternalInput").ap()
    ind_in = nc.dram_tensor("ind", [B_LOC, B_LOC * P], f32, kind="ExternalInput").ap()
    attn = nc.dram_tensor("attn", [B_LOC, S], f32, kind="ExternalOutput").ap()

    with tile.TileContext(nc) as tc:
        with (
            tc.tile_pool(name="consts", bufs=1) as consts,
            tc.tile_pool(name="fpa", bufs=4) as fpa,
            tc.tile_pool(name="fpb", bufs=4) as fpb,
            tc.tile_pool(name="qsb", bufs=2) as qsb,
            tc.tile_pool(name="smp", bufs=2) as smp,
            tc.tile_pool(name="qps", bufs=2, space="PSUM") as qps,
            tc.tile_pool(name="erps", bufs=2, space="PSUM") as erps,
        ):
            # gpsimd queue order matters at startup: q_rows first, then the
            # batch-0 broadcast (DVE is still idle, so gpsimd SBUF-write
            # contention is free), then ind + identity (needed much later).
            # q_rows first on the sync ring (high priority). Its completion is
            # still gated ~18us by straggler SDMA engines, which bounds how
            # early batch 0's broadcast can run. Keeping it off gpsimd/SWDGE
            # matters: a pending SWDGE DMA forces a long DRAIN before
            # PartitionBroadcast.
            q_rows = consts.tile([B_LOC, H], f32)
            with tc.high_priority():
                nc.sync.dma_start(out=q_rows[:], in_=questions)

            # ind[:, b*128:(b+1)*128] is the [8, 128] selector for batch b:
            # row b ones, rest zero -> matmul(ind_b, q_rows) broadcasts q[b].
            ind = consts.tile([B_LOC, B_LOC * P], f32)

            # energies: column b*SC+c holds energies[b, c*128:(c+1)*128] on partitions
            E = consts.tile([P, B_LOC * SC], f32)
            dummy = consts.tile([P, H], bf16)

            def emit_q_broadcast(b):
                """Broadcast q[b] to [128, H].

                Batch 0 uses gpsimd partition_broadcast: DVE is still idle at
                startup, so gpsimd's SBUF-write contention is free and q_b(0)
                is ready ~15 us earlier than the PE chain could deliver it.
                Later batches use PE outer-product + ACT copies via PSUM;
                gpsimd broadcast there would degrade the critical DVE op
                ~4.4 -> ~5.7 us (measured).
                """
                q_b = qsb.tile([P, H], f32)
                if b == 0:
                    # high priority: the scheduler otherwise defers this ~13us,
                    # and the first DVE op waits on it
                    with tc.high_priority():
                        nc.gpsimd.partition_broadcast(q_b[:], q_rows[0:1, :])
                    return q_b
                for k in range(H // NB):
                    ps = qps.tile([P, NB], f32)
                    nc.tensor.matmul(
                        ps[:],
                        ind[:, b * P : (b + 1) * P],
                        q_rows[:, k * NB : (k + 1) * NB],
                        start=True,
                        stop=True,
                    )
                    nc.scalar.copy(q_b[:, k * NB : (k + 1) * NB], ps[:])
                return q_b

            q_b0 = emit_q_broadcast(0)
            # ind (PE broadcast selector) is only needed ~22us in
            nc.sync.dma_start(out=ind[:], in_=ind_in)
            identity = consts.tile([P, P], f32)
            make_identity(nc, identity[:])

            def emit_group_softmax(g):
                """softmax + output store for batches [4g, 4g+4).

                PE transposes strided E column-slices straight into row layout:
                er_ps[b_g, c*128+i] = E[i, (4g+b_g)*SC+c], so no regroup DMA.
                """
                er_ps = erps.tile([SC, S], f32)
                Ev = E[:].rearrange("p (b c) -> p c b", c=SC)  # [128, c, b]
                for c in range(SC):
                    nc.tensor.transpose(
                        er_ps[:, c * P : (c + 1) * P],
                        Ev[:, c, 4 * g : 4 * g + 4],
                        identity[:],
                    )
                # max (DVE) and exp (ACT) read the PSUM tile directly; no copy
                nmax = smp.tile([SC, 1], f32)
                nc.vector.reduce_max(
                    nmax[:], er_ps[:], axis=mybir.AxisListType.X, negate=True
                )
                pexp = smp.tile([SC, S], f32)
                dn = smp.tile([SC, 1], f32)
                nc.scalar.activation(
                    pexp[:],
                    er_ps[:],
                    mybir.ActivationFunctionType.Exp,
                    bias=nmax[:],
                    scale=1.0,
                    accum_out=dn[:],
                )
                rc = smp.tile([SC, 1], f32)
                nc.vector.reciprocal(rc[:], dn[:])
                at = smp.tile([SC, S], f32)
                nc.vector.tensor_scalar_mul(at[:], pexp[:], rc[:])
                nc.scalar.dma_start(out=attn[4 * g : 4 * g + 4, :], in_=at[:])

            q_cur = q_b0
            del q_b0
            for b in range(B_LOC):
                ftiles = []
                for c in range(SC):
                    t = b * SC + c
                    pool, eng = (fpa, nc.sync) if t % 2 == 0 else (fpb, nc.scalar)
                    ftile = pool.tile([P, H], f32)
                    if t == 0:
                        # split the first tile across both rings so the first
                        # DVE op can start ~5us earlier
                        nc.sync.dma_start(
                            out=ftile[:, : H // 2],
                            in_=facts[b, c * P : (c + 1) * P, : H // 2],
                        )
                        nc.scalar.dma_start(
                            out=ftile[:, H // 2 :],
                            in_=facts[b, c * P : (c + 1) * P, H // 2 :],
                        )
                    else:
                        eng.dma_start(
                            out=ftile[:], in_=facts[b, c * P : (c + 1) * P, :]
                        )
                    ftiles.append(ftile)
                for c in range(SC):
                    col = b * SC + c
                    # fused multiply + row-sum on DVE; dummy bf16 out (values unused)
                    nc.vector.scalar_tensor_tensor(
                        out=dummy[:],
                        in0=ftiles[c][:],
                        scalar=1.0,
                        in1=q_cur[:],
                        op0=mybir.AluOpType.bypass,
                        op1=mybir.AluOpType.mult,
                        accum_out=E[:, col : col + 1],
                    )
                if b + 1 < B_LOC:
                    q_next = emit_q_broadcast(b + 1)
                else:
                    q_next = None
                if b == 3:
                    emit_group_softmax(0)
                q_cur = q_next
            emit_group_softmax(1)

    nc.compile()
    return nc


def _get_nc():
    if "nc" not in _CACHE:
        _CACHE["nc"] = _build_bass()
    return _CACHE["nc"]


def _shard_inputs(questions, facts):
    questions = np.ascontiguousarray(np.asarray(questions), dtype=np.float32)
    facts = np.ascontiguousarray(np.asarray(facts), dtype=np.float32)
    ind = np.zeros((B_LOC, B_LOC * P), dtype=np.float32)
    for b in range(B_LOC):
        ind[b, b * P : (b + 1) * P] = 1.0
    in_maps = []
    for i in range(N_CORES):
        sl = slice(i * B_LOC, (i + 1) * B_LOC)
        in_maps.append(
            {
                "facts": np.ascontiguousarray(facts[sl]),
                "questions": np.ascontiguousarray(questions[sl]),
                "ind": ind,
            }
        )
    return in_maps


def _run(questions, facts, **run_kwargs):
    from concourse.bass_utils import run_bass_kernel_spmd

    nc = _get_nc()
    in_maps = _shard_inputs(questions, facts)
    res = run_bass_kernel_spmd(nc, in_maps, core_ids=list(range(N_CORES)), **run_kwargs)
    out = np.stack([np.asarray(res.results[i]["attn"]) for i in range(N_CORES)])
    return out.reshape(B, S)[:, None, :].astype(np.float32), res


def kernel(questions, facts):
    out, _ = _run(questions, facts)
    return out
